# revision 28
# baseline (speedup 1.0000x reference)
"""MultiHeadAttention Trainium2 kernel (8 NeuronCores, SPMD).

Sharding: core c = (batch b=c//4, head-group g=c%4); each core owns 4 of 16
heads for one batch element. Wq/Wk/Wv are split by output features (tensor
parallel on heads), Wo by input features (row parallel); the 4 partial
[S, D] outputs per batch are summed on the host.

Matmul operands are bf16 (fp32 PSUM accumulation); softmax statistics and
normalization stay fp32. Layout is fully "transposed" (features on
partitions) so no on-chip transposes are needed anywhere:
  - QT/KT = projections in [feat, seq] layout (lhsT = host-transposed W)
  - scores^T[h] = KT_h^T-slices @ QT_h  (K=64 contraction, row-tiled pairs)
  - exp via ACT straight out of PSUM
  - ctx^T accumulated col-tiled per head pair; softmax denominators via an
    all-ones stationary matmul (partition reduction on the PE)
  - normalization deferred: ctx^T scaled by a DMA-partition-broadcast fp32
    reciprocal of the denominators
  - out projection contracts the core's 256 features; bias/bv folded on host
"""

import numpy as np

B, S, D = 2, 2048, 1024
H, DK = 16, 64
HG = 4                 # heads per core
FC = HG * DK           # 256 features per core
NCORES = 8
P = 128
KSUB = D // P          # 8 contraction subtiles for projections
FT = FC // P           # 2 feature tiles (= head pairs)
NKT = S // P           # 16 key-position tiles
QC = 512               # q-chunk size
NQC = S // QC          # 4
SCHUNK = 512           # s-chunk for streaming projections
NSC = S // SCHUNK      # 4

_PROGRAM = None        # cached Bass program - build once per process


def _build_program():
    from contextlib import ExitStack

    import concourse.bass as bass
    import concourse.mybir as mybir
    import concourse.tile as tile
    from concourse import bacc

    f32 = mybir.dt.float32
    bf16 = mybir.dt.bfloat16
    EXP = mybir.ActivationFunctionType.Exp
    IDENT = mybir.ActivationFunctionType.Identity

    nc = bacc.Bacc("TRN2", target_bir_lowering=False, debug=False)

    qT = nc.dram_tensor("qT", [D, S], bf16, kind="ExternalInput")
    kT = nc.dram_tensor("kT", [D, S], bf16, kind="ExternalInput")
    vT = nc.dram_tensor("vT", [D, S], bf16, kind="ExternalInput")
    wqT = nc.dram_tensor("wqT", [D, FC], bf16, kind="ExternalInput")
    wkT = nc.dram_tensor("wkT", [D, FC], bf16, kind="ExternalInput")
    wvT = nc.dram_tensor("wvT", [D, FC], bf16, kind="ExternalInput")
    woT = nc.dram_tensor("woT", [FC, D], bf16, kind="ExternalInput")
    bq = nc.dram_tensor("bq", [FC], f32, kind="ExternalInput")
    bk = nc.dram_tensor("bk", [FC], f32, kind="ExternalInput")
    out = nc.dram_tensor("out", [S, D], f32, kind="ExternalOutput")

    with tile.TileContext(nc) as tc, ExitStack() as ctx, nc.allow_low_precision(
        reason="bf16 matmul operands are intentional"
    ):
        weights = ctx.enter_context(tc.tile_pool(name="weights", bufs=1))
        instream = ctx.enter_context(tc.tile_pool(name="instream", bufs=5))
        persist = ctx.enter_context(tc.tile_pool(name="persist", bufs=1))
        exps = ctx.enter_context(tc.tile_pool(name="exps", bufs=4))
        ctpool = ctx.enter_context(tc.tile_pool(name="ctpool", bufs=2))
        outsb = ctx.enter_context(tc.tile_pool(name="outsb", bufs=2))
        small = ctx.enter_context(tc.tile_pool(name="small", bufs=3))
        ps_sc = ctx.enter_context(tc.tile_pool(name="ps_sc", bufs=2, space="PSUM"))
        ps_long = ctx.enter_context(tc.tile_pool(name="ps_long", bufs=4, space="PSUM"))
        drpool = ctx.enter_context(tc.tile_pool(name="drpool", bufs=2, space="DRAM"))

        # ---- persistent weights ----
        wq_sb = weights.tile([P, KSUB, FC], bf16, tag="wq")
        nc.sync.dma_start(wq_sb, wqT[:, :].rearrange("(o p) f -> p o f", p=P))
        wk_sb = weights.tile([P, KSUB, FC], bf16, tag="wk")
        nc.sync.dma_start(wk_sb, wkT[:, :].rearrange("(o p) f -> p o f", p=P))
        wv_sb = weights.tile([P, KSUB, FC], bf16, tag="wv")
        nc.sync.dma_start(wv_sb, wvT[:, :].rearrange("(o p) f -> p o f", p=P))
        wo_sb = weights.tile([P, FT, D], bf16, tag="wo")
        nc.sync.dma_start(wo_sb, woT[:, :].rearrange("(t p) j -> p t j", p=P))
        bq_sb = weights.tile([P, FT], f32, tag="bq")
        nc.sync.dma_start(bq_sb, bq[:].rearrange("(t p) -> p t", p=P))
        bk_sb = weights.tile([P, FT], f32, tag="bk")
        nc.sync.dma_start(bk_sb, bk[:].rearrange("(t p) -> p t", p=P))

        # ---- persistent activations ----
        QT = persist.tile([P, FT, S], bf16, tag="QT")   # [feat, seq]
        KT = persist.tile([P, FT, S], bf16, tag="KT")   # [feat, seq]
        V = persist.tile([P, NKT, HG, 66], bf16, tag="V")  # [seq, h, dk+ones]
        nc.vector.memset(V[:, :, :, 64:65], 1.0)

        # ---- projections, streamed over s-chunks ----
        qTr = qT[:, :].rearrange("(o p) s -> p o s", p=P)
        kTr = kT[:, :].rearrange("(o p) s -> p o s", p=P)
        vTr = vT[:, :].rearrange("(o p) s -> p o s", p=P)
        for c in range(NSC):
            sl = slice(c * SCHUNK, (c + 1) * SCHUNK)
            # K first: attention needs all of KT/V but only QT[:, :, 0:QC]
            kc = instream.tile([P, KSUB, SCHUNK], bf16, tag="instream")
            nc.sync.dma_start(kc, kTr[:, :, sl])
            for ft in range(FT):
                ps = ps_long.tile([P, SCHUNK], f32, tag="long")
                for ks in range(KSUB):
                    nc.tensor.matmul(
                        ps,
                        lhsT=wk_sb[:, ks, ft * P:(ft + 1) * P],
                        rhs=kc[:, ks, :],
                        start=(ks == 0),
                        stop=(ks == KSUB - 1),
                    )
                nc.vector.tensor_scalar_add(
                    out=KT[:, ft, sl], in0=ps, scalar1=bk_sb[:, ft:ft + 1]
                )
            vc = instream.tile([P, KSUB, SCHUNK], bf16, tag="instream")
            nc.sync.dma_start(vc, vTr[:, :, sl])
            for st in range(SCHUNK // P):
                ps = ps_long.tile([P, SCHUNK], f32, tag="long")
                for ks in range(KSUB):
                    nc.tensor.matmul(
                        ps[:, :FC],
                        lhsT=vc[:, ks, st * P:(st + 1) * P],
                        rhs=wv_sb[:, ks, :],
                        start=(ks == 0),
                        stop=(ks == KSUB - 1),
                    )
                for h in range(HG):
                    nc.vector.tensor_copy(
                        out=V[:, c * (SCHUNK // P) + st, h, 0:DK],
                        in_=ps[:, h * DK:(h + 1) * DK],
                    )
            qc_t = instream.tile([P, KSUB, SCHUNK], bf16, tag="instream")
            nc.sync.dma_start(qc_t, qTr[:, :, sl])
            for ft in range(FT):
                ps = ps_long.tile([P, SCHUNK], f32, tag="long")
                for ks in range(KSUB):
                    nc.tensor.matmul(
                        ps,
                        lhsT=wq_sb[:, ks, ft * P:(ft + 1) * P],
                        rhs=qc_t[:, ks, :],
                        start=(ks == 0),
                        stop=(ks == KSUB - 1),
                    )
                nc.vector.tensor_scalar_add(
                    out=QT[:, ft, sl], in0=ps, scalar1=bq_sb[:, ft:ft + 1]
                )

        # ---- attention + output projection, per q-chunk ----
        for qc in range(NQC):
            qsl = slice(qc * QC, (qc + 1) * QC)
            # one [65, QC] accumulator per head: rows 0:64 = unnormalized
            # ctx^T, row 64 = softmax denominator (from V's ones column)
            ctxu = [
                ps_long.tile([P, QC], f32, tag="long", name=f"ctxu{hh}")
                for hh in range(HG)
            ]
            ex = [None, None]
            for kt in range(NKT):
                ksl = slice(kt * P, (kt + 1) * P)
                first, last = kt == 0, kt == NKT - 1
                for ft in range(FT):
                    sc = ps_sc.tile([P, 2 * QC], f32, tag="sc")
                    # scores^T for the head pair, row-tiled (K=64 each)
                    nc.tensor.matmul(
                        sc[:, 0:QC],
                        lhsT=KT[0:64, ft, ksl],
                        rhs=QT[0:64, ft, qsl],
                        start=True, stop=True,
                        tile_position=(0, 0),
                    )
                    nc.tensor.matmul(
                        sc[:, QC:2 * QC],
                        lhsT=KT[64:128, ft, ksl],
                        rhs=QT[64:128, ft, qsl],
                        start=True, stop=True,
                        tile_position=(64, 0),
                    )
                    e = exps.tile([P, 2 * QC], bf16, tag="exps")
                    nc.scalar.activation(e, sc, EXP)
                    ex[ft] = e
                # ctx^T + denominator accumulation (augmented V, M=65)
                for h in range(HG):
                    nc.tensor.matmul(
                        ctxu[h][0:65, :],
                        lhsT=V[:, kt, h, 0:65],
                        rhs=ex[h // 2][:, (h % 2) * QC:(h % 2 + 1) * QC],
                        start=first, stop=last,
                    )

            # epilogue: normalize ctx^T (fp32) and run the output projection.
            # Denominators sit on partition 64 of each head's accumulator; a
            # DRAM bounce + partition-step-0 DMA broadcasts each reciprocal
            # row across the 64 partitions of that head's ctx^T, in fp32.
            recip = small.tile([P, HG * QC], f32, tag="recip")
            for h in range(HG):
                nc.vector.reciprocal(
                    recip[64:65, h * QC:(h + 1) * QC], ctxu[h][64:65, :]
                )
            dr = drpool.tile([1, HG * QC], f32, tag="dr")
            nc.sync.dma_start(dr, recip[64:65, :])
            ct = ctpool.tile([P, FT, QC], bf16, tag="ct")
            for h in range(HG):
                ft, half = divmod(h, 2)
                bcs = small.tile([64, QC], f32, tag="bcs")
                row = dr[0:1, h * QC:(h + 1) * QC]
                bsrc = bass.AP(
                    tensor=row.tensor,
                    offset=row.offset,
                    ap=[[0, 64]] + [list(x) for x in row.ap[1:]],
                )
                nc.gpsimd.dma_start(bcs, bsrc)
                if half == 0:
                    nc.vector.tensor_mul(
                        out=ct[0:64, ft, :], in0=ctxu[h][0:64, :], in1=bcs
                    )
                else:
                    cttmp = small.tile([64, QC], bf16, tag="cttmp")
                    nc.vector.tensor_mul(
                        out=cttmp, in0=ctxu[h][0:64, :], in1=bcs
                    )
                    nc.sync.dma_start(ct[64:128, ft, :], cttmp)

            for st in range(QC // P):
                s0 = qc * QC + st * P
                osb = outsb.tile([P, D], f32, tag="osb")
                for jc in range(D // 512):
                    ops = ps_long.tile([P, QC], f32, tag="long", name="ops")
                    for ft in range(FT):
                        nc.tensor.matmul(
                            ops,
                            lhsT=ct[:, ft, st * P:(st + 1) * P],
                            rhs=wo_sb[:, ft, jc * 512:(jc + 1) * 512],
                            start=(ft == 0),
                            stop=(ft == FT - 1),
                        )
                    nc.vector.tensor_copy(out=osb[:, jc * 512:(jc + 1) * 512], in_=ops)
                nc.sync.dma_start(out[s0:s0 + P, :], osb)

    nc.compile()
    return nc


def _get_program():
    global _PROGRAM
    if _PROGRAM is None:
        _PROGRAM = _build_program()
    return _PROGRAM


def _host_shards(q, k, v, Wq, bq, Wk, bk, Wv, bv, Wo, bo):
    """Build the 8 per-core input dicts (host-side transposes/slices)."""
    import ml_dtypes

    b16 = ml_dtypes.bfloat16
    scale = 1.0 / np.sqrt(np.float32(DK))
    qT = [np.ascontiguousarray(q[b].T).astype(b16) for b in range(B)]
    kT = [np.ascontiguousarray(k[b].T).astype(b16) for b in range(B)]
    vT = [np.ascontiguousarray(v[b].T).astype(b16) for b in range(B)]
    in_maps = []
    for c in range(NCORES):
        b, g = divmod(c, NCORES // B)
        fsl = slice(g * FC, (g + 1) * FC)
        in_maps.append({
            "qT": qT[b],
            "kT": kT[b],
            "vT": vT[b],
            "wqT": np.ascontiguousarray(Wq[fsl, :].T * scale).astype(b16),
            "wkT": np.ascontiguousarray(Wk[fsl, :].T).astype(b16),
            "wvT": np.ascontiguousarray(Wv[fsl, :].T).astype(b16),
            "woT": np.ascontiguousarray(Wo[:, fsl].T).astype(b16),
            "bq": np.ascontiguousarray(bq[fsl] * scale),
            "bk": np.ascontiguousarray(bk[fsl]),
        })
    return in_maps


def kernel(q, k, v, mask, Wq, bq, Wk, bk, Wv, bv, Wo, bo):
    q = np.asarray(q, dtype=np.float32)
    k = np.asarray(k, dtype=np.float32)
    v = np.asarray(v, dtype=np.float32)
    mask = np.asarray(mask)
    Wq = np.asarray(Wq, dtype=np.float32)
    bq = np.asarray(bq, dtype=np.float32)
    Wk = np.asarray(Wk, dtype=np.float32)
    bk = np.asarray(bk, dtype=np.float32)
    Wv = np.asarray(Wv, dtype=np.float32)
    bv = np.asarray(bv, dtype=np.float32)
    Wo = np.asarray(Wo, dtype=np.float32)
    bo = np.asarray(bo, dtype=np.float32)

    if not np.all(mask != 0):
        # Unmasked-path kernel; fall back to exact host computation if a
        # nontrivial mask ever shows up (spec fills the mask with ones).
        return _host_reference(q, k, v, mask, Wq, bq, Wk, bk, Wv, bv, Wo, bo)

    from concourse.bass_utils import run_bass_kernel_spmd

    nc = _get_program()
    in_maps = _host_shards(q, k, v, Wq, bq, Wk, bk, Wv, bv, Wo, bo)
    res = run_bass_kernel_spmd(nc, in_maps, core_ids=list(range(NCORES)))

    # host reduction: sum the 4 row-parallel Wo partials per batch,
    # then add the exact bv/bo correction (softmax rows sum to 1).
    const = bv @ Wo.T + bo
    out = np.empty((B, S, D), np.float32)
    gpb = NCORES // B
    for b in range(B):
        acc = res.results[b * gpb]["out"].astype(np.float32)
        for g in range(1, gpb):
            acc = acc + res.results[b * gpb + g]["out"]
        out[b] = acc + const[None, :]
    return out


def _host_reference(q, k, v, mask, Wq, bq, Wk, bk, Wv, bv, Wo, bo):
    def split_heads(x):
        b, s, _ = x.shape
        return x.reshape(b, s, H, DK).transpose(0, 2, 1, 3)

    query = split_heads(q @ Wq.T + bq)
    key_ = split_heads(k @ Wk.T + bk)
    value = split_heads(v @ Wv.T + bv)
    scores = np.einsum("bhqd,bhkd->bhqk", query, key_) / np.sqrt(np.float32(DK))
    scores = np.where(mask == 0, np.float32(-1e9), scores).astype(np.float32)
    scores -= scores.max(axis=-1, keepdims=True)
    e = np.exp(scores)
    attn = e / e.sum(axis=-1, keepdims=True)
    ctx = np.einsum("bhqk,bhkd->bhqd", attn, value)
    ctx = ctx.transpose(0, 2, 1, 3).reshape(q.shape[0], -1, D)
    return (ctx @ Wo.T + bo).astype(np.float32)
